# revision 12
# baseline (speedup 1.0000x reference)
"""ApsPool3d TRN2 kernel v2.

Per core (1 batch): input (64, 48, 48, 48) f32 -> output (64, 24, 24, 24).

Design:
- Input via gpsimd SWDGE cast-DMA (f32 DRAM -> bf16 SBUF, ~427GB/s read).
- y-blur: two single-op box2 passes on DVE (zero-guard layout).
- z-blur (x) [1,2,1] + x-blur [1,2,1] via 18 PE matmuls/tile (weights reordered
  side*12 then cent*6 -> 2 weight loads per tile).
- PSUM evac on ScalarE into phase-major (deinterleaved y/x parity) bf16 layout.
- Norm squares batched per 4 tiles, phase-contiguous, split DVE/ScalarE.
- Phase argmax -> registers; z-parity one-hot SEL matrix via cond-DMA;
  extraction = 64 small matmuls (SEL^T @ stored-slice with dynamic offset),
  evac to bf16 outbuf, 4 big output DMAs. Host reassembles + casts f32.

STAGE env gates debug dumps (0: cast, 1: y-blur, 2: stored, 3: norms, 5: full).
"""

import os
import sys

for _p in ("/opt/trn_rl_repo", "/root/.axon_site/_ro/trn_rl_repo"):
    if _p not in sys.path:
        sys.path.insert(0, _p)

import numpy as np

import concourse.bass as bass
import concourse.mybir as mybir
import concourse.tile as tile


# ---- inlined tile patch: drain/barrier carrier fix ----
def _patched_drain_and_barrier(self, tick_clock, wait_clock):
    nc = self.nc
    carrier = mybir.InstNoOp(
        name="tile_drain_wait_carrier",
        engine=mybir.EngineType.SP,
        ins=[],
        outs=[],
    )
    wait_clock.add_sem_waits(
        carrier, tile.ScopedClock({None: tick_clock.global_clock})
    )
    waits = list(carrier.sync_info.on_wait) if carrier.sync_info else []
    for w in waits:
        nop = nc.sync.nop()
        nsi = nop.ins.sync_info
        if nsi is None:
            nop.ins.sync_info = mybir.SyncInfo(on_wait=[w], on_update=[])
        else:
            nsi.on_wait.append(w)
    nc.sync.drain()
    nc.all_engine_barrier()
    assert self.sems is not None
    popped = nc._tile_sem_poison_stack.pop()
    assert popped is self._sem_poison
    nc.clear_and_free_semaphores(list(self.sems.allocated().values()))
    nc.all_engine_barrier()


tile.TileContext._drain_and_barrier = _patched_drain_and_barrier

_SPLIT_SEQ = [0]


def _split_waits(nc, max_waits=1):
    for f in nc.m.functions:
        for bb in f.blocks:
            new_insts = []
            for inst in bb.instructions:
                si = inst.sync_info
                if si is not None and si.on_wait and len(si.on_wait) > max_waits:
                    waits = list(si.on_wait)
                    keep = waits[:max_waits]
                    extras = waits[max_waits:]
                    del si.on_wait[:]
                    si.on_wait.extend(keep)
                    for w in extras:
                        _SPLIT_SEQ[0] += 1
                        nop = mybir.InstNoOp(
                            name=f"waitsplit-{_SPLIT_SEQ[0]}",
                            engine=inst.engine,
                            ins=[],
                            outs=[],
                            sync_info=mybir.SyncInfo(on_wait=[w], on_update=[]),
                        )
                        new_insts.append(nop)
                new_insts.append(inst)
            if len(new_insts) != len(bb.instructions):
                del bb.instructions[:]
                bb.instructions.extend(new_insts)


def build_weights(filt):
    """W_side/W_cent (96,96) f32 (z-blur x x-tap weights) and parity P (96,2)."""
    f = np.asarray(filt[0, 0], dtype=np.float64)
    s = f.sum()
    sz = f.sum(axis=(1, 2)) / s
    sy = f.sum(axis=(0, 2)) / s
    sx = f.sum(axis=(0, 1)) / s
    assert abs(sx[0] - sx[2]) < 1e-12 and abs(sy[0] - sy[2]) < 1e-12
    zp = [2 * i for i in range(24)] + [2 * i + 1 for i in range(24)]
    blk_side = np.zeros((48, 48), dtype=np.float64)
    blk_cent = np.zeros((48, 48), dtype=np.float64)
    for m in range(48):
        z_out = zp[m]
        for dz in (-1, 0, 1):
            z_in = z_out + dz
            if 0 <= z_in < 48:
                w = sz[dz + 1] * sy[0]
                blk_side[z_in, m] = w * sx[0]
                blk_cent[z_in, m] = w * sx[1]
    W_side = np.zeros((96, 96), dtype=np.float64)
    W_cent = np.zeros((96, 96), dtype=np.float64)
    for c in range(2):
        W_side[c * 48 : c * 48 + 48, c * 48 : c * 48 + 48] = blk_side
        W_cent[c * 48 : c * 48 + 48, c * 48 : c * 48 + 48] = blk_cent
    P = np.zeros((96, 2), dtype=np.float32)
    for c in range(2):
        P[c * 48 : c * 48 + 24, 0] = 1.0
        P[c * 48 + 24 : c * 48 + 48, 1] = 1.0
    return W_side.astype(np.float32), W_cent.astype(np.float32), P



from concourse.bass_utils import run_bass_kernel_spmd

F32 = mybir.dt.float32
BF16 = mybir.dt.bfloat16
FP8 = mybir.dt.float8e4
U32 = mybir.dt.uint32
ALU = mybir.AluOpType

C, N = 64, 48
NH = N // 2  # 24
YX = N * N  # 2304
NT = C // 2  # 32 channel-pair tiles
ROWP = 50
GPAD = 48  # guard cols each side of input tile

STAGE = int(os.environ.get("STAGE", "5"))


def build_sel_mats():
    """P_even / P_odd (96, 48) bf16 one-hot z-parity selectors."""
    pe = np.zeros((96, 48), dtype=np.float32)
    po = np.zeros((96, 48), dtype=np.float32)
    for cl in range(2):
        for z in range(NH):
            pe[cl * N + z, cl * NH + z] = 1.0
            po[cl * N + NH + z, cl * NH + z] = 1.0
    return pe, po


def build_kernel(nc):
    x = nc.declare_dram_parameter("x", [C, N, N, N], F32, isOutput=False)
    w_side_d = nc.declare_dram_parameter("w_side", [128, 128], BF16, isOutput=False)
    w_cent_d = nc.declare_dram_parameter("w_cent", [128, 128], BF16, isOutput=False)
    par_d = nc.declare_dram_parameter("par", [96, 2], F32, isOutput=False)
    p_even_d = nc.declare_dram_parameter("p_even", [96, 48], BF16, isOutput=False)
    p_odd_d = nc.declare_dram_parameter("p_odd", [96, 48], BF16, isOutput=False)
    out = nc.declare_dram_parameter("out", [48, NT * 576], BF16, isOutput=True)

    dbg16 = dbg32 = dbgidx = None
    if STAGE == 0:
        dbg16 = nc.declare_dram_parameter("dbg16", [96, 2400], BF16, isOutput=True)
    if STAGE == 1:
        dbg16 = nc.declare_dram_parameter("dbg16", [96, 2404], BF16, isOutput=True)
    if STAGE == 2:
        dbg16 = nc.declare_dram_parameter("dbg16", [96, 4608], BF16, isOutput=True)
    if STAGE == 3:
        dbg32 = nc.declare_dram_parameter("dbg32", [1, 8], F32, isOutput=True)
        dbgidx = nc.declare_dram_parameter("dbgidx", [1, 8], U32, isOutput=True)

    xf = x.rearrange("c z y x -> (c z) (y x)")  # (3072, 2304)

    with tile.TileContext(nc) as tc:
        with (
            tc.tile_pool(name="consts", bufs=1) as consts,
            tc.tile_pool(name="inp", bufs=1) as inp_pool,
            tc.tile_pool(name="work", bufs=1) as work_pool,
            tc.tile_pool(name="ps", bufs=1, space="PSUM") as psum_pool,
            tc.tile_pool(name="store", bufs=1) as store_pool,
            tc.tile_pool(name="stg", bufs=1) as stage_pool,
        ):
            w_side = consts.tile([128, 128], BF16, tag="ws")
            w_cent = consts.tile([128, 128], BF16, tag="wc")
            par = consts.tile([96, 2], F32, tag="par")
            p_even = consts.tile([96, 48], BF16, tag="pe")
            nc.sync.dma_start(w_side[:], w_side_d[:])
            nc.sync.dma_start(w_cent[:], w_cent_d[:])
            nc.sync.dma_start(par[:], par_d[:])
            nc.sync.dma_start(p_even[:], p_even_d[:])

            stored = store_pool.tile([96, NT * YX + 80], BF16, tag="stored")
            sel = consts.tile([96, 48], BF16, tag="sel")
            nc.vector.tensor_copy(sel[:], p_even[:])
            norm_acc = consts.tile([128, 96], F32, tag="nacc")

            its = [
                inp_pool.tile([96, 2400], BF16, tag=f"it{i}", name=f"it{i}")
                for i in range(3)
            ]
            t1s = [
                work_pool.tile([96, 2352], BF16, tag=f"t1_{i}", name=f"t1_{i}")
                for i in range(1)
            ]
            us = [
                work_pool.tile([128, 2404], BF16, tag=f"u_{i}", name=f"u_{i}")
                for i in range(2)
            ]
            junks = {
                "v": work_pool.tile([128, 2304], FP8, tag="junk_v", name="junk_v"),
                "s": work_pool.tile([128, 2304], FP8, tag="junk_s", name="junk_s"),
            }
            for i in range(3):
                gv = its[i][0:96, 0:2400].rearrange(
                    "p (s q) -> p s q", s=50
                )  # unused view trick avoided; memset guards directly
                nc.vector.memset(its[i][0:96, 0:GPAD], 0.0)
                nc.vector.memset(its[i][0:96, GPAD + YX : 2400], 0.0)
            for i in range(2):
                nc.vector.memset(us[i][:], 0.0)

            psums = [
                psum_pool.tile([128, 1536], F32, tag=f"ps_{i}", name=f"ps_{i}")
                for i in range(2)
            ]


            # phase-parity (pc = dx*2 + dy) -> block offset dx*1152 + dy*24
            pc_off = [0, 24, 1152, 1176]

            # warmup: tiny SWDGE transfer pays the cold-path cost early
            nc.gpsimd.dma_start(its[2][0:1, GPAD : GPAD + 128], xf[0:1, 0:128])

            for t in range(NT):
                it = its[t % 3]

                # ---- input: SWDGE cast-DMA f32 -> bf16 ----
                nc.gpsimd.dma_start(
                    it[0:96, GPAD : GPAD + YX], xf[96 * t : 96 * (t + 1), :]
                )

                if STAGE == 0:
                    if t == 0:
                        nc.sync.dma_start(dbg16[:], it[:])
                        break
                    continue

                t1 = t1s[0]
                u = us[t % 2]

                # ---- y box2 #1: t1[r'] = d[r'-1] + d[r'], r' in [0,49) ----
                nc.vector.tensor_add(
                    t1[:, 0:2352], it[0:96, 0:2352], it[0:96, GPAD : GPAD + 2352]
                )
                # ---- y box2 #2 into ROWP=50 layout ----
                uv = u[0:96, 0:2400].rearrange("p (r w) -> p r w", w=ROWP)
                nc.vector.tensor_add(
                    uv[:, :, 2:50],
                    t1[:, 0:2304].rearrange("p (r w) -> p r w", w=N),
                    t1[:, 48:2352].rearrange("p (r w) -> p r w", w=N),
                )

                if STAGE == 1:
                    if t == 0:
                        nc.sync.dma_start(dbg16[:], u[0:96, 0:2404])
                        break
                    continue

                # ---- PE: 12 side then 6 cent matmuls (chunked, <=512 psum) ----
                def rhs_for(h, c, off):
                    r0 = (h * 3 + c) * 8
                    a = ROWP * r0 + off
                    return u[0:128, a : a + ROWP * 8].rearrange(
                        "p (r w) -> p r w", w=ROWP
                    )[:, :, 0:48]

                for h in range(2):
                    pv = psums[h][0:128, 0:1536].rearrange("p (c w) -> p c w", c=3)
                    for c in range(3):
                        pout = pv[:, c, 0:384]
                        nc.tensor.matmul(
                            pout, w_side[:], rhs_for(h, c, 1),
                            start=True, stop=False, skip_group_check=True,
                        )
                        nc.tensor.matmul(
                            pout, w_side[:], rhs_for(h, c, 3),
                            start=False, stop=False, skip_group_check=True,
                        )
                for h in range(2):
                    pv = psums[h][0:128, 0:1536].rearrange("p (c w) -> p c w", c=3)
                    for c in range(3):
                        pout = pv[:, c, 0:384]
                        nc.tensor.matmul(
                            pout, w_cent[:], rhs_for(h, c, 2),
                            start=False, stop=True, skip_group_check=True,
                        )

                # ---- evac: ScalarE, one op per half (contiguous dst) ----
                sblk = stored[0:96, t * YX : (t + 1) * YX].rearrange(
                    "p (hh q) -> p hh q", hh=2
                )
                for h in range(2):
                    psv = (
                        psums[h][0:96, 0:1536]
                        .rearrange("p (c q) -> p c q", c=3)[:, :, 0:384]
                    )
                    dst = sblk[:, h].rearrange("p (c q) -> p c q", c=3)
                    nc.scalar.copy(dst, psv)

                if STAGE == 2:
                    if t == 1:
                        nc.sync.dma_start(dbg16[:], stored[0:96, 0:4608])
                        break
                    continue

                # ---- norm squares: V per tile, S per 4-tile group ----
                gg, tts = t // 4, t % 4
                sv1 = stored[0:96, t * YX : (t + 1) * YX].rearrange(
                    "p (y x) -> p y x", y=48
                )
                jv2 = junks["v"][0:96, 0:576].rearrange(
                    "p (yh xh) -> p yh xh", yh=24
                )
                for pc in range(2):
                    v1 = sv1[:, pc & 1 : N : 2, (pc >> 1) : N : 2]
                    col = pc * 32 + gg * 4 + tts
                    nc.vector.scalar_tensor_tensor(
                        jv2, v1, 1.0, v1, ALU.bypass, ALU.mult,
                        accum_out=norm_acc[0:96, col : col + 1],
                    )
                if t % 2 == 1:
                    half = (t % 4) // 2
                    sv2 = stored[0:96, (t - 1) * YX : (t + 1) * YX].rearrange(
                        "p (tt y x) -> p tt y x", tt=2, y=48
                    )
                    for pc in range(2, 4):
                        xp_, yp_ = pc >> 1, pc & 1
                        v = sv2[:, :, yp_ : N : 2, xp_ : N : 2]
                        jv = junks["s"][0:96, 0:1152].rearrange(
                            "p (tt yh xh) -> p tt yh xh", tt=2, yh=24
                        )
                        col = 64 + (pc - 2) * 16 + gg * 2 + half
                        nc.scalar.activation(
                            jv, v,
                            mybir.ActivationFunctionType.Square,
                            accum_out=norm_acc[0:96, col : col + 1],
                        )

            if STAGE <= 2:
                return

            # ---- finalize norms ----
            zred = psums[1][0:2, 0:96]
            nc.tensor.matmul(
                zred, par[:, 0:2], norm_acc[0:96, :],
                start=True, stop=True, skip_group_check=True,
            )
            zred_s = consts.tile([2, 96], F32, tag="zreds")
            nc.scalar.copy(zred_s[:], zred)
            # flip (2,96) -> (1,192) via SBUF->SBUF DMA
            nbig = consts.tile([1, 192], F32, tag="nbig")
            nc.sync.dma_start(
                nbig[0:1, 0:192].rearrange("o (p f) -> o p f", p=2),
                zred_s[:],
            )
            norms8 = consts.tile([1, 8], F32, tag="norms8")
            nzb = nbig[0:1, 0:192].rearrange("o (zb c) -> o zb c", zb=2)
            for pc, (c0, cn) in enumerate([(0, 32), (32, 32), (64, 16), (80, 16)]):
                nc.vector.tensor_reduce(
                    norms8[0:1, pc : pc + 5 : 4],
                    nzb[:, :, c0 : c0 + cn],
                    mybir.AxisListType.X, ALU.add,
                )
            nmax = consts.tile([1, 8], F32, tag="nmax")
            nidx = consts.tile([1, 8], U32, tag="nidx")
            nc.vector.max(nmax[:], norms8[:])
            nc.vector.max_index(nidx[:], nmax[:], norms8[:])

            if STAGE == 3:
                nc.sync.dma_start(dbg32[:], norms8[:])
                nc.sync.dma_start(dbgidx[:], nidx[:])
                return

            # ---- registers: phase -> block offset + z parity ----
            rp = nc.alloc_registers("rp")
            rblk = nc.alloc_registers("rblk")
            rz = nc.alloc_registers("rz")
            rtmp = nc.alloc_registers("rtmp")
            nc.regs_load(rp, nidx[0:1, 0:1])
            nc.regs_alu(rtmp, rp, 1, ALU.bitwise_and)  # dy
            nc.regs_alu(rblk, rtmp, 48, ALU.mult)
            nc.regs_alu(rtmp, rp, 1, ALU.logical_shift_right)
            nc.regs_alu(rtmp, rtmp, 1, ALU.bitwise_and)  # dx
            nc.regs_alu(rblk, rblk, rtmp, ALU.add)
            nc.regs_alu(rtmp, rp, 2, ALU.logical_shift_right)
            nc.regs_alu(rz, rtmp, 1, ALU.bitwise_and)  # dz
            rz_s = nc.snap(rz, min_val=0, max_val=1)
            blk_off = nc.snap(rblk, min_val=0, max_val=49)

            # ---- SEL matrix: P_even (prebuilt), P_odd overwrite if dz ----
            nc.sync.dma_start(sel[:], p_odd_d[:], cond=rz_s)

            # ---- extraction: dyn-copy 4 tiles -> static matmuls ----
            outbufs = [
                stage_pool.tile([48, 4 * 576], BF16, tag=f"ob{i}", name=f"ob{i}")
                for i in range(2)
            ]
            stgs = [
                stage_pool.tile([96, 4 * 576], BF16, tag=f"sg{i}", name=f"sg{i}")
                for i in range(4)
            ]
            for t in range(NT):
                if t % 4 == 0:
                    g = t // 4
                    stg = stgs[g % 4]
                    sv = (
                        stored[0:96, g * 4 * YX : g * 4 * YX + 4 * YX + 64][
                            :, bass.ds(blk_off, 4 * YX)
                        ]
                        .rearrange("p (tt y x) -> p tt y x", tt=4, y=48)[
                            :, :, 0:48:2, 0:48:2
                        ]
                    )
                    dstv = stg[:].rearrange(
                        "p (tt yh xh) -> p tt yh xh", tt=4, yh=24
                    )
                    if g % 2 == 0:
                        nc.vector.tensor_copy(dstv, sv)
                    else:
                        nc.scalar.copy(dstv, sv)
                if t % 4 != 3:
                    continue
                g = t // 4
                stg = stgs[g % 4]
                ob = outbufs[g % 2]
                # 5 matmuls: 4x512 + 1x256 cols over the group's 2304 cols
                for k in range(5):
                    c0 = 512 * k
                    cn = 512 if k < 4 else 256
                    if k < 3:
                        pdst = psums[0][0:48, c0 : c0 + cn]
                    else:
                        pdst = psums[1][0:48, c0 - 1536 : c0 - 1536 + cn]
                    nc.tensor.matmul(
                        pdst, sel[:], stg[:, c0 : c0 + cn],
                        start=True, stop=True, skip_group_check=True,
                    )
                # 2 evacs: psums[0][0:1536] and psums[1][0:768]
                if g % 2 == 0:
                    nc.vector.tensor_copy(ob[0:48, 0:1536], psums[0][0:48, 0:1536])
                    nc.scalar.copy(ob[0:48, 1536:2304], psums[1][0:48, 0:768])
                else:
                    nc.scalar.copy(ob[0:48, 0:1536], psums[0][0:48, 0:1536])
                    nc.vector.tensor_copy(ob[0:48, 1536:2304], psums[1][0:48, 0:768])
                nc.sync.dma_start(
                    out[0:48, g * 2304 : (g + 1) * 2304], ob[:]
                )


_NC_CACHE = {}


def _get_nc():
    key = STAGE
    if key not in _NC_CACHE:
        nc = bass.Bass()
        build_kernel(nc)
        _split_waits(nc)
        _NC_CACHE[key] = nc
    return _NC_CACHE[key]


def run(input_to_pool, filt, trace=False):
    import ml_dtypes

    W_side, W_cent, P = build_weights(np.asarray(filt))
    W_side = np.pad(W_side, ((0, 32), (0, 32)))  # 128x128 -> FWL eligible
    W_cent = np.pad(W_cent, ((0, 32), (0, 32)))
    pe, po = build_sel_mats()
    nc = _get_nc()
    x = np.ascontiguousarray(np.asarray(input_to_pool, dtype=np.float32))
    B = x.shape[0]
    in_maps = []
    for b in range(B):
        in_maps.append(
            {
                "x": x[b],
                "w_side": W_side.astype(ml_dtypes.bfloat16),
                "w_cent": W_cent.astype(ml_dtypes.bfloat16),
                "par": P,
                "p_even": pe.astype(ml_dtypes.bfloat16),
                "p_odd": po.astype(ml_dtypes.bfloat16),
            }
        )
    res = run_bass_kernel_spmd(nc, in_maps, core_ids=list(range(B)), trace=trace)
    return res


def assemble(out_flat):
    """(48, 32*576) bf16 -> (64, 24, 24, 24) f32."""
    a = np.asarray(out_flat).astype(np.float32)
    a = a.reshape(2, 24, 32, 24, 24)  # [cl, z, t, yh, xh]
    a = np.transpose(a, (2, 0, 1, 3, 4))  # [t, cl, z, yh, xh]
    return a.reshape(64, 24, 24, 24)


def kernel(input_to_pool, filt, permute_indices=None):
    res = run(input_to_pool, filt, trace=False)
    B = np.asarray(input_to_pool).shape[0]
    outs = np.stack([assemble(res.results[b]["out"]) for b in range(B)], axis=0)
    return outs


# revision 15
# speedup vs baseline: 1.0008x; 1.0008x over previous
"""ApsPool3d TRN2 kernel v2.

Per core (1 batch): input (64, 48, 48, 48) f32 -> output (64, 24, 24, 24).

Design:
- Input via gpsimd SWDGE cast-DMA (f32 DRAM -> bf16 SBUF, ~427GB/s read).
- y-blur: two single-op box2 passes on DVE (zero-guard layout).
- z-blur (x) [1,2,1] + x-blur [1,2,1] via 18 PE matmuls/tile (weights reordered
  side*12 then cent*6 -> 2 weight loads per tile).
- PSUM evac on ScalarE into phase-major (deinterleaved y/x parity) bf16 layout.
- Norm squares batched per 4 tiles, phase-contiguous, split DVE/ScalarE.
- Phase argmax -> registers; z-parity one-hot SEL matrix via cond-DMA;
  extraction = 64 small matmuls (SEL^T @ stored-slice with dynamic offset),
  evac to bf16 outbuf, 4 big output DMAs. Host reassembles + casts f32.

STAGE env gates debug dumps (0: cast, 1: y-blur, 2: stored, 3: norms, 5: full).
"""

import os
import sys

for _p in ("/opt/trn_rl_repo", "/root/.axon_site/_ro/trn_rl_repo"):
    if _p not in sys.path:
        sys.path.insert(0, _p)

import numpy as np

import concourse.bass as bass
import concourse.mybir as mybir
import concourse.tile as tile


# ---- inlined tile patch: drain/barrier carrier fix ----
def _patched_drain_and_barrier(self, tick_clock, wait_clock):
    nc = self.nc
    carrier = mybir.InstNoOp(
        name="tile_drain_wait_carrier",
        engine=mybir.EngineType.SP,
        ins=[],
        outs=[],
    )
    wait_clock.add_sem_waits(
        carrier, tile.ScopedClock({None: tick_clock.global_clock})
    )
    waits = list(carrier.sync_info.on_wait) if carrier.sync_info else []
    for w in waits:
        nop = nc.sync.nop()
        nsi = nop.ins.sync_info
        if nsi is None:
            nop.ins.sync_info = mybir.SyncInfo(on_wait=[w], on_update=[])
        else:
            nsi.on_wait.append(w)
    nc.sync.drain()
    nc.all_engine_barrier()
    assert self.sems is not None
    popped = nc._tile_sem_poison_stack.pop()
    assert popped is self._sem_poison
    nc.clear_and_free_semaphores(list(self.sems.allocated().values()))
    nc.all_engine_barrier()


tile.TileContext._drain_and_barrier = _patched_drain_and_barrier

_SPLIT_SEQ = [0]


def _split_waits(nc, max_waits=1):
    for f in nc.m.functions:
        for bb in f.blocks:
            new_insts = []
            for inst in bb.instructions:
                si = inst.sync_info
                if si is not None and si.on_wait and len(si.on_wait) > max_waits:
                    waits = list(si.on_wait)
                    keep = waits[:max_waits]
                    extras = waits[max_waits:]
                    del si.on_wait[:]
                    si.on_wait.extend(keep)
                    for w in extras:
                        _SPLIT_SEQ[0] += 1
                        nop = mybir.InstNoOp(
                            name=f"waitsplit-{_SPLIT_SEQ[0]}",
                            engine=inst.engine,
                            ins=[],
                            outs=[],
                            sync_info=mybir.SyncInfo(on_wait=[w], on_update=[]),
                        )
                        new_insts.append(nop)
                new_insts.append(inst)
            if len(new_insts) != len(bb.instructions):
                del bb.instructions[:]
                bb.instructions.extend(new_insts)


def build_weights(filt):
    """W_side/W_cent (96,96) f32 (z-blur x x-tap weights) and parity P (96,2)."""
    f = np.asarray(filt[0, 0], dtype=np.float64)
    s = f.sum()
    sz = f.sum(axis=(1, 2)) / s
    sy = f.sum(axis=(0, 2)) / s
    sx = f.sum(axis=(0, 1)) / s
    assert abs(sx[0] - sx[2]) < 1e-12 and abs(sy[0] - sy[2]) < 1e-12
    zp = [2 * i for i in range(24)] + [2 * i + 1 for i in range(24)]
    blk_side = np.zeros((48, 48), dtype=np.float64)
    blk_cent = np.zeros((48, 48), dtype=np.float64)
    for m in range(48):
        z_out = zp[m]
        for dz in (-1, 0, 1):
            z_in = z_out + dz
            if 0 <= z_in < 48:
                w = sz[dz + 1] * sy[0]
                blk_side[z_in, m] = w * sx[0]
                blk_cent[z_in, m] = w * sx[1]
    W_side = np.zeros((96, 96), dtype=np.float64)
    W_cent = np.zeros((96, 96), dtype=np.float64)
    for c in range(2):
        W_side[c * 48 : c * 48 + 48, c * 48 : c * 48 + 48] = blk_side
        W_cent[c * 48 : c * 48 + 48, c * 48 : c * 48 + 48] = blk_cent
    P = np.zeros((96, 2), dtype=np.float32)
    for c in range(2):
        P[c * 48 : c * 48 + 24, 0] = 1.0
        P[c * 48 + 24 : c * 48 + 48, 1] = 1.0
    return W_side.astype(np.float32), W_cent.astype(np.float32), P



from concourse.bass_utils import run_bass_kernel_spmd

F32 = mybir.dt.float32
BF16 = mybir.dt.bfloat16
FP8 = mybir.dt.float8e4
U32 = mybir.dt.uint32
ALU = mybir.AluOpType

C, N = 64, 48
NH = N // 2  # 24
YX = N * N  # 2304
NT = C // 2  # 32 channel-pair tiles
ROWP = 50
GPAD = 48  # guard cols each side of input tile

STAGE = int(os.environ.get("STAGE", "5"))


def build_sel_mats():
    """P_even / P_odd (96, 48) bf16 one-hot z-parity selectors."""
    pe = np.zeros((96, 48), dtype=np.float32)
    po = np.zeros((96, 48), dtype=np.float32)
    for cl in range(2):
        for z in range(NH):
            pe[cl * N + z, cl * NH + z] = 1.0
            po[cl * N + NH + z, cl * NH + z] = 1.0
    return pe, po


def build_kernel(nc):
    x = nc.declare_dram_parameter("x", [C, N, N, N], F32, isOutput=False)
    w_side_d = nc.declare_dram_parameter("w_side", [128, 128], BF16, isOutput=False)
    w_cent_d = nc.declare_dram_parameter("w_cent", [128, 128], BF16, isOutput=False)
    par_d = nc.declare_dram_parameter("par", [96, 2], F32, isOutput=False)
    p_even_d = nc.declare_dram_parameter("p_even", [96, 48], BF16, isOutput=False)
    p_odd_d = nc.declare_dram_parameter("p_odd", [96, 48], BF16, isOutput=False)
    out = nc.declare_dram_parameter("out", [48, NT * 576], BF16, isOutput=True)

    dbg16 = dbg32 = dbgidx = None
    if STAGE == 0:
        dbg16 = nc.declare_dram_parameter("dbg16", [96, 2400], BF16, isOutput=True)
    if STAGE == 1:
        dbg16 = nc.declare_dram_parameter("dbg16", [96, 2404], BF16, isOutput=True)
    if STAGE == 2:
        dbg16 = nc.declare_dram_parameter("dbg16", [96, 4608], BF16, isOutput=True)
    if STAGE == 3:
        dbg32 = nc.declare_dram_parameter("dbg32", [1, 8], F32, isOutput=True)
        dbgidx = nc.declare_dram_parameter("dbgidx", [1, 8], U32, isOutput=True)

    xf = x.rearrange("c z y x -> (c z) (y x)")  # (3072, 2304)

    with tile.TileContext(nc) as tc:
        with (
            tc.tile_pool(name="consts", bufs=1) as consts,
            tc.tile_pool(name="inp", bufs=1) as inp_pool,
            tc.tile_pool(name="work", bufs=1) as work_pool,
            tc.tile_pool(name="ps", bufs=1, space="PSUM") as psum_pool,
            tc.tile_pool(name="store", bufs=1) as store_pool,
            tc.tile_pool(name="stg", bufs=1) as stage_pool,
        ):
            w_side = consts.tile([128, 128], BF16, tag="ws")
            w_cent = consts.tile([128, 128], BF16, tag="wc")
            par = consts.tile([96, 2], F32, tag="par")
            p_even = consts.tile([96, 48], BF16, tag="pe")
            nc.sync.dma_start(w_side[:], w_side_d[:])
            nc.sync.dma_start(w_cent[:], w_cent_d[:])
            nc.sync.dma_start(par[:], par_d[:])
            nc.sync.dma_start(p_even[:], p_even_d[:])

            stored = store_pool.tile([96, NT * YX + 80], BF16, tag="stored")
            sel = consts.tile([96, 48], BF16, tag="sel")
            nc.vector.tensor_copy(sel[:], p_even[:])
            norm_acc = consts.tile([128, 80], F32, tag="nacc")

            its = [
                inp_pool.tile([96, 2400], BF16, tag=f"it{i}", name=f"it{i}")
                for i in range(3)
            ]
            t1s = [
                work_pool.tile([96, 2352], BF16, tag=f"t1_{i}", name=f"t1_{i}")
                for i in range(1)
            ]
            us = [
                work_pool.tile([128, 2404], BF16, tag=f"u_{i}", name=f"u_{i}")
                for i in range(2)
            ]
            junks = {
                "v": work_pool.tile([128, 2304], FP8, tag="junk_v", name="junk_v"),
                "s": work_pool.tile([128, 2304], FP8, tag="junk_s", name="junk_s"),
            }
            for i in range(3):
                gv = its[i][0:96, 0:2400].rearrange(
                    "p (s q) -> p s q", s=50
                )  # unused view trick avoided; memset guards directly
                nc.vector.memset(its[i][0:96, 0:GPAD], 0.0)
                nc.vector.memset(its[i][0:96, GPAD + YX : 2400], 0.0)
            for i in range(2):
                nc.vector.memset(us[i][:], 0.0)

            psums = [
                psum_pool.tile([128, 1536], F32, tag=f"ps_{i}", name=f"ps_{i}")
                for i in range(2)
            ]


            # phase-parity (pc = dx*2 + dy) -> block offset dx*1152 + dy*24
            pc_off = [0, 24, 1152, 1176]

            # warmup: tiny SWDGE transfer pays the cold-path cost early
            nc.gpsimd.dma_start(its[2][0:1, GPAD : GPAD + 128], xf[0:1, 0:128])

            for t in range(NT):
                it = its[t % 3]

                # ---- input: SWDGE cast-DMA f32 -> bf16 ----
                nc.gpsimd.dma_start(
                    it[0:96, GPAD : GPAD + YX], xf[96 * t : 96 * (t + 1), :]
                )

                if STAGE == 0:
                    if t == 0:
                        nc.sync.dma_start(dbg16[:], it[:])
                        break
                    continue

                t1 = t1s[0]
                u = us[t % 2]

                # ---- y box2 #1: t1[r'] = d[r'-1] + d[r'], r' in [0,49) ----
                nc.vector.tensor_add(
                    t1[:, 0:2352], it[0:96, 0:2352], it[0:96, GPAD : GPAD + 2352]
                )
                # ---- y box2 #2 into ROWP=50 layout ----
                uv = u[0:96, 0:2400].rearrange("p (r w) -> p r w", w=ROWP)
                nc.vector.tensor_add(
                    uv[:, :, 2:50],
                    t1[:, 0:2304].rearrange("p (r w) -> p r w", w=N),
                    t1[:, 48:2352].rearrange("p (r w) -> p r w", w=N),
                )

                if STAGE == 1:
                    if t == 0:
                        nc.sync.dma_start(dbg16[:], u[0:96, 0:2404])
                        break
                    continue

                # ---- PE: 12 side then 6 cent matmuls (chunked, <=512 psum) ----
                def rhs_for(h, c, off):
                    r0 = (h * 3 + c) * 8
                    a = ROWP * r0 + off
                    return u[0:128, a : a + ROWP * 8].rearrange(
                        "p (r w) -> p r w", w=ROWP
                    )[:, :, 0:48]

                for h in range(2):
                    pv = psums[h][0:128, 0:1536].rearrange("p (c w) -> p c w", c=3)
                    for c in range(3):
                        pout = pv[:, c, 0:384]
                        nc.tensor.matmul(
                            pout, w_side[:], rhs_for(h, c, 1),
                            start=True, stop=False, skip_group_check=True,
                        )
                        nc.tensor.matmul(
                            pout, w_side[:], rhs_for(h, c, 3),
                            start=False, stop=False, skip_group_check=True,
                        )
                for h in range(2):
                    pv = psums[h][0:128, 0:1536].rearrange("p (c w) -> p c w", c=3)
                    for c in range(3):
                        pout = pv[:, c, 0:384]
                        nc.tensor.matmul(
                            pout, w_cent[:], rhs_for(h, c, 2),
                            start=False, stop=True, skip_group_check=True,
                        )

                # ---- evac: ScalarE, one op per half (contiguous dst) ----
                sblk = stored[0:96, t * YX : (t + 1) * YX].rearrange(
                    "p (hh q) -> p hh q", hh=2
                )
                for h in range(2):
                    psv = (
                        psums[h][0:96, 0:1536]
                        .rearrange("p (c q) -> p c q", c=3)[:, :, 0:384]
                    )
                    dst = sblk[:, h].rearrange("p (c q) -> p c q", c=3)
                    nc.scalar.copy(dst, psv)

                if STAGE == 2:
                    if t == 1:
                        nc.sync.dma_start(dbg16[:], stored[0:96, 0:4608])
                        break
                    continue

                # ---- norm squares: V per tile, S per 4-tile group ----
                gg, tts = t // 4, t % 4
                sv1 = stored[0:96, t * YX : (t + 1) * YX].rearrange(
                    "p (y x) -> p y x", y=48
                )
                jv2 = junks["v"][0:96, 0:576].rearrange(
                    "p (yh xh) -> p yh xh", yh=24
                )
                for pc in range(2):
                    v1 = sv1[:, pc & 1 : N : 2, (pc >> 1) : N : 2]
                    col = pc * 32 + gg * 4 + tts
                    nc.vector.scalar_tensor_tensor(
                        jv2, v1, 1.0, v1, ALU.bypass, ALU.mult,
                        accum_out=norm_acc[0:96, col : col + 1],
                    )
                if t % 4 == 3:
                    sv4 = stored[0:96, (t - 3) * YX : (t + 1) * YX].rearrange(
                        "p (tt y x) -> p tt y x", tt=4, y=48
                    )
                    for pc in range(2, 4):
                        xp_, yp_ = pc >> 1, pc & 1
                        v = sv4[:, :, yp_ : N : 2, xp_ : N : 2]
                        jv = junks["s"][0:96].rearrange(
                            "p (tt yh xh) -> p tt yh xh", tt=4, yh=24
                        )
                        col = 64 + (pc - 2) * 8 + gg
                        nc.scalar.activation(
                            jv, v,
                            mybir.ActivationFunctionType.Square,
                            accum_out=norm_acc[0:96, col : col + 1],
                        )

            if STAGE <= 2:
                return

            # ---- finalize norms ----
            zred = psums[1][0:2, 0:80]
            nc.tensor.matmul(
                zred, par[:, 0:2], norm_acc[0:96, :],
                start=True, stop=True, skip_group_check=True,
            )
            zred_s = consts.tile([2, 80], F32, tag="zreds")
            nc.scalar.copy(zred_s[:], zred)
            # flip (2,80) -> (1,160) via SBUF->SBUF DMA
            nbig = consts.tile([1, 160], F32, tag="nbig")
            nc.sync.dma_start(
                nbig[0:1, 0:160].rearrange("o (p f) -> o p f", p=2),
                zred_s[:],
            )
            norms8 = consts.tile([1, 8], F32, tag="norms8")
            nzb = nbig[0:1, 0:160].rearrange("o (zb c) -> o zb c", zb=2)
            for pc, (c0, cn) in enumerate([(0, 32), (32, 32), (64, 8), (72, 8)]):
                nc.vector.tensor_reduce(
                    norms8[0:1, pc : pc + 5 : 4],
                    nzb[:, :, c0 : c0 + cn],
                    mybir.AxisListType.X, ALU.add,
                )
            nmax = consts.tile([1, 8], F32, tag="nmax")
            nidx = consts.tile([1, 8], U32, tag="nidx")
            nc.vector.max(nmax[:], norms8[:])
            nc.vector.max_index(nidx[:], nmax[:], norms8[:])

            if STAGE == 3:
                nc.sync.dma_start(dbg32[:], norms8[:])
                nc.sync.dma_start(dbgidx[:], nidx[:])
                return

            # ---- registers: phase -> block offset + z parity ----
            rp = nc.alloc_registers("rp")
            rblk = nc.alloc_registers("rblk")
            rz = nc.alloc_registers("rz")
            rtmp = nc.alloc_registers("rtmp")
            nc.regs_load(rp, nidx[0:1, 0:1])
            nc.regs_alu(rtmp, rp, 1, ALU.bitwise_and)  # dy
            nc.regs_alu(rblk, rtmp, 48, ALU.mult)
            nc.regs_alu(rtmp, rp, 1, ALU.logical_shift_right)
            nc.regs_alu(rtmp, rtmp, 1, ALU.bitwise_and)  # dx
            nc.regs_alu(rblk, rblk, rtmp, ALU.add)
            nc.regs_alu(rtmp, rp, 2, ALU.logical_shift_right)
            nc.regs_alu(rz, rtmp, 1, ALU.bitwise_and)  # dz
            rz_s = nc.snap(rz, min_val=0, max_val=1)
            blk_off = nc.snap(rblk, min_val=0, max_val=49)

            # ---- SEL matrix: P_even (prebuilt), P_odd overwrite if dz ----
            nc.sync.dma_start(sel[:], p_odd_d[:], cond=rz_s)

            # ---- extraction: dyn-copy 4 tiles -> static matmuls ----
            outbufs = [
                stage_pool.tile([48, 4 * 576], BF16, tag=f"ob{i}", name=f"ob{i}")
                for i in range(2)
            ]
            stgs = [
                stage_pool.tile([96, 4 * 576], BF16, tag=f"sg{i}", name=f"sg{i}")
                for i in range(4)
            ]
            for t in range(NT):
                if t % 4 == 0:
                    g = t // 4
                    stg = stgs[g % 4]
                    sv = (
                        stored[0:96, g * 4 * YX : g * 4 * YX + 4 * YX + 64][
                            :, bass.ds(blk_off, 4 * YX)
                        ]
                        .rearrange("p (tt y x) -> p tt y x", tt=4, y=48)[
                            :, :, 0:48:2, 0:48:2
                        ]
                    )
                    dstv = stg[:].rearrange(
                        "p (tt yh xh) -> p tt yh xh", tt=4, yh=24
                    )
                    if g % 2 == 0 or g == 7:
                        nc.vector.tensor_copy(dstv, sv)
                    else:
                        nc.scalar.copy(dstv, sv)
                if t % 4 != 3:
                    continue
                g = t // 4
                stg = stgs[g % 4]
                ob = outbufs[g % 2]
                # 5 matmuls: 4x512 + 1x256 cols over the group's 2304 cols
                for k in range(5):
                    c0 = 512 * k
                    cn = 512 if k < 4 else 256
                    if k < 3:
                        pdst = psums[0][0:48, c0 : c0 + cn]
                    else:
                        pdst = psums[1][0:48, c0 - 1536 : c0 - 1536 + cn]
                    nc.tensor.matmul(
                        pdst, sel[:], stg[:, c0 : c0 + cn],
                        start=True, stop=True, skip_group_check=True,
                    )
                # 2 evacs: psums[0][0:1536] and psums[1][0:768]
                if g % 2 == 0:
                    nc.vector.tensor_copy(ob[0:48, 0:1536], psums[0][0:48, 0:1536])
                    nc.scalar.copy(ob[0:48, 1536:2304], psums[1][0:48, 0:768])
                else:
                    nc.scalar.copy(ob[0:48, 0:1536], psums[0][0:48, 0:1536])
                    nc.vector.tensor_copy(ob[0:48, 1536:2304], psums[1][0:48, 0:768])
                nc.sync.dma_start(
                    out[0:48, g * 2304 : (g + 1) * 2304], ob[:]
                )


_NC_CACHE = {}


def _get_nc():
    key = STAGE
    if key not in _NC_CACHE:
        nc = bass.Bass()
        build_kernel(nc)
        _split_waits(nc)
        _NC_CACHE[key] = nc
    return _NC_CACHE[key]


def run(input_to_pool, filt, trace=False):
    import ml_dtypes

    W_side, W_cent, P = build_weights(np.asarray(filt))
    W_side = np.pad(W_side, ((0, 32), (0, 32)))  # 128x128 -> FWL eligible
    W_cent = np.pad(W_cent, ((0, 32), (0, 32)))
    pe, po = build_sel_mats()
    nc = _get_nc()
    x = np.ascontiguousarray(np.asarray(input_to_pool, dtype=np.float32))
    B = x.shape[0]
    in_maps = []
    for b in range(B):
        in_maps.append(
            {
                "x": x[b],
                "w_side": W_side.astype(ml_dtypes.bfloat16),
                "w_cent": W_cent.astype(ml_dtypes.bfloat16),
                "par": P,
                "p_even": pe.astype(ml_dtypes.bfloat16),
                "p_odd": po.astype(ml_dtypes.bfloat16),
            }
        )
    res = run_bass_kernel_spmd(nc, in_maps, core_ids=list(range(B)), trace=trace)
    return res


def assemble(out_flat):
    """(48, 32*576) bf16 -> (64, 24, 24, 24) f32."""
    a = np.asarray(out_flat).astype(np.float32)
    a = a.reshape(2, 24, 32, 24, 24)  # [cl, z, t, yh, xh]
    a = np.transpose(a, (2, 0, 1, 3, 4))  # [t, cl, z, yh, xh]
    return a.reshape(64, 24, 24, 24)


def kernel(input_to_pool, filt, permute_indices=None):
    res = run(input_to_pool, filt, trace=False)
    B = np.asarray(input_to_pool).shape[0]
    outs = np.stack([assemble(res.results[b]["out"]) for b in range(B)], axis=0)
    return outs
